# revision 2
# baseline (speedup 1.0000x reference)
"""Trainium2 Bass kernel for sparse voxel convolution (nn_Convolution_59717225284148).

Math: each input point p contributes features[p] @ kernel[p mod 2] to the output
row of its even-floor anchor cell (2*(p//2)).  The conv is therefore
8 parity-grouped matmuls + a cell-indexed scatter-add.

Distribution: points are sorted by cell, cells are range-partitioned across the
8 cores (balanced by point count) -> per-core outputs are disjoint row ranges,
no collectives needed.

Per-core device kernel (SPMD, one program):
  phase 1: for each parity slab, matmul  Y[tok,256] = X[tok,128ch] @ W[parity]
           (lhsT = transposed feature chunk, rhs = weight; PSUM tile lands in
           exactly the token-wrapped layout dma_scatter_add consumes)
  phase 2: dma_scatter_add Y rows into out[cell_local] (HBM CCE-add).
           Indices are unique within each (parity, cell-quarter) call by
           construction (host pre-merges duplicate positions); calls that can
           conflict (same cell from different parities) share a queue and are
           chained with explicit deps; different queues own disjoint cell
           quarters so they never conflict.

Host side: index computation, duplicate-position merge (conv is linear in
features), feature transpose/padding, and final unshard (concat row ranges).
"""
import os
import functools

import numpy as np

try:
    import concourse.bacc as bacc  # noqa: F401
except Exception:  # pragma: no cover
    import sys
    for _p in ("/opt/trn_rl_repo", "/root/.axon_site/_ro/trn_rl_repo"):
        if _p not in sys.path:
            sys.path.append(_p)
    import concourse.bacc as bacc

import concourse.mybir as mybir
import concourse.tile as tile
from concourse.tile import add_dep_helper
from concourse import library_config
from concourse.bass_utils import run_bass_kernel_spmd

N_CORES = 8
NSLAB = 8          # parity groups (kernel taps)
IN_CH = 128
OUT_CH = 256

_DT_MAP = {
    "f32": mybir.dt.float32,
    "f32r": mybir.dt.float32r,
    "bf16": mybir.dt.bfloat16,
}


def _dt_np(dt):
    return mybir.dt.np(dt)


@functools.lru_cache(maxsize=8)
def _build_program(sq: int, m_pad: int, nq: int, dt_key: str):
    """Build + compile the SPMD Bass program.

    sq:    padded tokens per (parity, queue) group (multiple of 128)
    m_pad: output rows per core
    nq:    scatter queues (cell-range quarters)
    """
    dt_mm = _DT_MAP[dt_key]
    s_slab = nq * sq                  # tokens per parity slab
    t_pad = NSLAB * s_slab            # total padded tokens per core
    nch_q = sq // 128                 # chunks per (d,q) group
    nch_slab = s_slab // 128

    nc = bacc.Bacc("TRN2", target_bir_lowering=False, debug=False,
                   num_devices=N_CORES, num_swdge_queues=nq)
    xt = nc.dram_tensor("xt", [128, t_pad], dt_mm, kind="ExternalInput").ap()
    w = nc.dram_tensor("w", [128, NSLAB * OUT_CH], dt_mm, kind="ExternalInput").ap()
    idx = nc.dram_tensor("idx", [128, t_pad // 16], mybir.dt.int16,
                         kind="ExternalInput").ap()
    meta = nc.dram_tensor("meta", [1, 64], mybir.dt.int32, kind="ExternalInput").ap()
    out = nc.dram_tensor("out", [m_pad, OUT_CH], mybir.dt.float32,
                         kind="ExternalOutput").ap()

    with tile.TileContext(nc) as tc:
        with (
            tc.tile_pool(name="wp", bufs=1) as wp,
            tc.tile_pool(name="xp", bufs=3) as xp,
            tc.tile_pool(name="yp", bufs=1) as yp,
            tc.tile_pool(name="ip", bufs=1) as ip,
            tc.tile_pool(name="mp", bufs=1) as mp,
            tc.tile_pool(name="pp", bufs=8, space="PSUM") as pp,
        ):
            nc.gpsimd.load_library(library_config.mlp)
            w_tile = wp.tile([128, NSLAB * OUT_CH], dt_mm)
            nc.sync.dma_start(out=w_tile[:], in_=w[:, :])
            idx_tile = ip.tile([128, t_pad // 16], mybir.dt.int16)
            nc.sync.dma_start(out=idx_tile[:], in_=idx[:, :])
            meta_tile = mp.tile([1, 64], mybir.dt.int32)
            nc.sync.dma_start(out=meta_tile[:], in_=meta[:, :])
            regs = [nc.gpsimd.alloc_register(f"cnt_q{q}") for q in range(nq)]

            y_tile = yp.tile([128, NSLAB * nch_slab * OUT_CH], mybir.dt.float32)
            for d in range(NSLAB):
                x_slab = xp.tile([128, s_slab], dt_mm, tag="xslab")
                nc.sync.dma_start(out=x_slab[:],
                                  in_=xt[:, d * s_slab:(d + 1) * s_slab])
                for c in range(nch_slab):
                    ps = pp.tile([128, OUT_CH], mybir.dt.float32, tag="ps")
                    nc.tensor.matmul(
                        out=ps[:],
                        lhsT=x_slab[:, c * 128:(c + 1) * 128],
                        rhs=w_tile[:, d * OUT_CH:(d + 1) * OUT_CH],
                        start=True, stop=True,
                    )
                    ycol = (d * nch_slab + c) * OUT_CH
                    nc.vector.tensor_copy(out=y_tile[:, ycol:ycol + OUT_CH],
                                          in_=ps[:])

            y_ap = y_tile[:].rearrange("p (c e) -> p c e", e=OUT_CH)
            fq = sq // 16
            prev = [None] * nq
            for d in range(NSLAB):
                for q in range(nq):
                    g = d * nq + q
                    ld = nc.gpsimd.reg_load(regs[q], meta_tile[0:1, g:g + 1])
                    if prev[q] is not None:
                        add_dep_helper(ld.ins, prev[q].ins, True,
                                       "count reg WAR after prior scatter")
                    inst = nc.gpsimd.dma_scatter_add(
                        out_ap=out[:, :],
                        in_ap=y_ap[:, g * nch_q:(g + 1) * nch_q, :],
                        idxs_ap=idx_tile[:, g * fq:(g + 1) * fq],
                        num_idxs=sq,
                        num_idxs_reg=regs[q],
                        elem_size=OUT_CH,
                        queue_num=q,
                    )
                    if prev[q] is not None:
                        add_dep_helper(inst.ins, prev[q].ins, True,
                                       "serialize conflicting scatters")
                    prev[q] = inst
    nc.compile()
    return nc


def _ceil_to(x, m):
    return int(-(-x // m) * m)


def _preprocess(features, inp_positions, nq):
    """Sort/merge points, plan the per-core layouts."""
    pos = np.asarray(inp_positions).astype(np.int64)
    feats = np.asarray(features, dtype=np.float32)

    par = pos & 1
    d_idx = (par[:, 0] << 2) | (par[:, 1] << 1) | par[:, 2]
    cell = pos >> 1
    ckey = (cell[:, 0] * 64 + cell[:, 1]) * 64 + cell[:, 2]
    poskey = (ckey << 3) | d_idx

    order = np.argsort(poskey, kind="stable")
    sk = poskey[order]
    new_tok = np.r_[True, sk[1:] != sk[:-1]]
    starts = np.flatnonzero(new_tok)
    fm = np.add.reduceat(feats[order], starts, axis=0)  # merged features [T,128]
    tk = sk[starts]
    t_ckey = tk >> 3
    t_d = (tk & 7).astype(np.int32)
    T = len(tk)

    cell_new = np.r_[True, t_ckey[1:] != t_ckey[:-1]]
    t_crank = np.cumsum(cell_new) - 1
    M = int(t_crank[-1]) + 1
    ucell = t_ckey[cell_new]

    cellcnt = np.bincount(t_crank, minlength=M)
    cum0 = np.r_[0, np.cumsum(cellcnt)]
    csplit = [0] + [int(np.searchsorted(cum0[1:], (T * (c + 1)) // N_CORES))
                    for c in range(N_CORES - 1)] + [M]

    plans = []
    max_ng = 0
    max_mc = 0
    for c in range(N_CORES):
        clo, chi = csplit[c], csplit[c + 1]
        mc = chi - clo
        tlo, thi = int(cum0[clo]), int(cum0[chi])
        tc_n = thi - tlo
        crl = (t_crank[tlo:thi] - clo).astype(np.int64)
        dl = t_d[tlo:thi].astype(np.int64)
        # queue quarters: split local cells so token counts are balanced
        ccnt_l = cellcnt[clo:chi]
        cuml = np.cumsum(ccnt_l)
        qb = [int(np.searchsorted(cuml, (tc_n * (i + 1)) // nq))
              for i in range(nq - 1)]
        qarr = np.searchsorted(np.asarray(qb, np.int64), crl, side="right")
        group = dl * nq + qarr
        go = np.argsort(group, kind="stable")
        ng = np.bincount(group, minlength=NSLAB * nq).astype(np.int32)
        plans.append(dict(mc=mc, tlo=tlo, thi=thi, crl=crl, go=go, ng=ng,
                          group=group))
        max_ng = max(max_ng, int(ng.max()))
        max_mc = max(max_mc, mc)

    sq = max(128, _ceil_to(max_ng, 128))
    m_pad = max(256, _ceil_to(max_mc, 256))
    return dict(fm=fm, T=T, M=M, ucell=ucell, plans=plans, sq=sq, m_pad=m_pad,
                csplit=csplit)


def _core_inputs(plan, fm, sq, nq, w_arr, np_dt):
    """Build xt/idx/meta arrays for one core."""
    t_pad = NSLAB * nq * sq
    tlo, thi = plan["tlo"], plan["thi"]
    tc_n = thi - tlo
    go = plan["go"]
    group = plan["group"]
    ng = plan["ng"]

    sorted_groups = group[go]
    gstart = np.r_[0, np.cumsum(ng)[:-1]]  # start of each group in sorted order
    within = np.arange(tc_n, dtype=np.int64) - gstart[sorted_groups]
    dest = sorted_groups * sq + within

    xo = np.zeros((t_pad, IN_CH), np.float32)
    xo[dest] = fm[tlo:thi][go]
    xt = np.ascontiguousarray(xo.T).astype(np_dt)

    idx_full = np.full(t_pad, -1, np.int16)
    idx_full[dest] = plan["crl"][go].astype(np.int16)
    # wrap each (d,q) window: token i of window -> [i%16, i//16]
    idx_w = idx_full.reshape(NSLAB * nq, sq // 16, 16)
    idx_w = np.transpose(idx_w, (0, 2, 1)).reshape(NSLAB * nq, 16, sq // 16)
    idx_16 = np.concatenate(list(idx_w), axis=1)  # [16, t_pad//16]
    idx_arr = np.tile(idx_16, (8, 1))             # replicate to 128 partitions

    meta = np.zeros((1, 64), np.int32)
    meta[0, :NSLAB * nq] = ng
    return {"xt": xt, "w": w_arr, "idx": idx_arr, "meta": meta}


def _run(features, inp_positions, kernel, trace=False):
    dt_key = os.environ.get("BASS_CONV_DT", "f32")
    nq = int(os.environ.get("BASS_CONV_NQ", "4"))
    np_dt = _dt_np(_DT_MAP[dt_key])

    pre = _preprocess(features, inp_positions, nq)
    sq, m_pad = pre["sq"], pre["m_pad"]

    kern = np.asarray(kernel, np.float32).reshape(NSLAB, IN_CH, OUT_CH)
    w_arr = np.ascontiguousarray(
        np.transpose(kern, (1, 0, 2)).reshape(IN_CH, NSLAB * OUT_CH)
    ).astype(np_dt)

    in_maps = [_core_inputs(p, pre["fm"], sq, nq, w_arr, np_dt)
               for p in pre["plans"]]

    nc = _build_program(sq, m_pad, nq, dt_key)
    res = run_bass_kernel_spmd(nc, in_maps, list(range(N_CORES)), trace=trace)

    out_full = np.concatenate(
        [res.results[c]["out"][:pre["plans"][c]["mc"]] for c in range(N_CORES)],
        axis=0,
    )
    ucell = pre["ucell"]
    anchors = np.stack([ucell >> 12, (ucell >> 6) & 63, ucell & 63], axis=1) * 2
    out_positions = (anchors.astype(np.float32) + 0.5) / 2.0
    return out_full, out_positions, res


def kernel(features, inp_positions, kernel):
    out_full, out_positions, _ = _run(features, inp_positions, kernel)
    return out_full, out_positions
